# revision 2
# baseline (speedup 1.0000x reference)
"""Multi-head self-attention (B=4, N=2048, D=768, H=12, dh=64) on 8 Trainium2
NeuronCores — v2.

Sharding: core c handles batch b = c // 2 and heads [6*(c%2), 6*(c%2)+6).
Host sums the two partial w_o products per batch element and adds b_o.

Key changes vs v1 (all bf16; fp8 rejected by numerics, rel err > 2e-2):
  * Score matmuls use zero-padded 128-row stationaries (KS) so the moving
    operand streams all 128 partitions: HW-measured 247ns vs 491ns for the
    64-partition form.
  * bf16 PE transposes of x (1 cy/row instead of 2 for f32).
  * Score groups of 2 kv-tiles (se/so tags, 2 banks each) + O_e/O_o = 6 PSUM
    banks, leaving 2 banks (pj tag) for projection / phase-D fills that
    double-buffer independently of the attention accumulators.
  * AV matmuls interleaved with next group's score matmuls (same-bank
    back-to-back PSUM accumulation measured +25% stall).
  * Epilogue normalizes straight out of PSUM (no staging copy).
"""
import sys

if "/opt/trn_rl_repo" not in sys.path:
    sys.path.insert(0, "/opt/trn_rl_repo")

import numpy as np

import concourse.bass as bass
import concourse.tile as tile
from concourse import bacc, mybir
from concourse.masks import make_identity

P = 128
B, N, D = 4, 2048, 768
HEADS, DH = 12, 64
HL = 6                 # heads per core
INNER_L = HL * DH      # 384 local inner dim
DC = D // P            # 6 chunks of model dim
IC = INNER_L // P      # 3 chunks of local inner dim (head pairs)
NT = N // P            # 16 token tiles
NQ = 512               # query chunk
QC = N // NQ           # 4 query chunks
G = 2                  # kv tiles per exp group
NG = NT // G           # groups per head

F32 = mybir.dt.float32
BF = mybir.dt.bfloat16

_CACHED_NC = None


def build_program(reps=1, loop_n=0):
    nc = bacc.Bacc("TRN2", target_bir_lowering=False, debug=False)

    x_d = nc.dram_tensor("x", [N, D], F32, kind="ExternalInput").ap()
    wq_d = nc.dram_tensor("w_q", [D, INNER_L], F32, kind="ExternalInput").ap()
    wk_d = nc.dram_tensor("w_k", [D, INNER_L], F32, kind="ExternalInput").ap()
    wv_d = nc.dram_tensor("w_v", [D, INNER_L], F32, kind="ExternalInput").ap()
    wo_d = nc.dram_tensor("w_o", [INNER_L, D], F32, kind="ExternalInput").ap()
    out_d = nc.dram_tensor("out", [N, D], F32, kind="ExternalOutput").ap()

    with tile.TileContext(nc) as tc:
        if loop_n:
            with tc.For_i(0, loop_n, 1):
                _build_body(nc, tc, 0, x_d, wq_d, wk_d, wv_d, wo_d, out_d)
        else:
            for rep in range(reps):
                _build_body(nc, tc, rep, x_d, wq_d, wk_d, wv_d, wo_d, out_d)
    nc.compile()
    return nc


def _build_body(nc, tc, rep, x_d, wq_d, wk_d, wv_d, wo_d, out_d):
    with tc.tile_pool(name=f"persist{rep}", bufs=1) as persist:
        xT = persist.tile([P, DC, N], BF)          # x^T, D on partitions
        QT = persist.tile([P, IC, N], BF)          # Q^T stacked head pairs
        KS = persist.tile([P, IC, NT, 2, P], BF)   # zero-padded K stationaries
        V = persist.tile([P, NT, HL, DH + 1], BF)  # V + ones col (denom)
        HT = persist.tile([P, IC, N], BF)          # normalized heads^T
        wq_sb = persist.tile([P, DC, INNER_L], BF)
        wk_sb = persist.tile([P, DC, INNER_L], BF)
        wv_sb = persist.tile([P, DC, INNER_L], BF)
        wo_sb = persist.tile([P, IC, D], BF)
        ident = persist.tile([P, P], BF)

        make_identity(nc, ident)
        nc.gpsimd.memset(KS, 0.0)
        nc.vector.memset(V[:, :, :, DH:DH + 1], 1.0)

        # ---- Phase A: load weights + x, cast to bf16, transpose x ----
        with (
            tc.tile_pool(name=f"stageA{rep}", bufs=1) as stageA,
            tc.tile_pool(name=f"wstage{rep}", bufs=2) as wstage,
            tc.tile_pool(name=f"psA{rep}", bufs=1, space="PSUM") as psA,
        ):
            x_f32 = stageA.tile([P, NT, D], F32)
            x_bf = stageA.tile([P, NT, D], BF)
            for ch in range(4):
                nc.sync.dma_start(
                    x_f32[:, 4 * ch:4 * ch + 4, :],
                    x_d.rearrange("(kt p) d -> p kt d", p=P)[:, 4 * ch:4 * ch + 4, :],
                )
            # weight casts on gpsimd so the DVE can focus on x casts +
            # transpose drains (shortens the critical prefix)
            for w_d_ap, w_sb in ((wq_d, wq_sb), (wk_d, wk_sb), (wv_d, wv_sb)):
                w_f32 = wstage.tile([P, DC, INNER_L], F32, tag="wf")
                nc.sync.dma_start(w_f32, w_d_ap.rearrange("(c p) i -> p c i", p=P))
                nc.gpsimd.tensor_copy(w_sb, w_f32)
            wo_f32 = wstage.tile([P, IC, D], F32, tag="wf")
            nc.sync.dma_start(wo_f32, wo_d.rearrange("(c p) o -> p c o", p=P))
            nc.gpsimd.tensor_copy(wo_sb, wo_f32)

            for ch in range(4):
                nc.vector.tensor_copy(
                    x_bf[:, 4 * ch:4 * ch + 4, :], x_f32[:, 4 * ch:4 * ch + 4, :]
                )
                for kt in range(4 * ch, 4 * ch + 4):
                    for c in range(DC):
                        tp = psA.tile([P, P], BF, tag=f"tp{c % 4}")
                        nc.tensor.transpose(
                            tp, x_bf[:, kt, c * P:(c + 1) * P], ident
                        )
                        nc.vector.tensor_copy(xT[:, c, kt * P:(kt + 1) * P], tp)

        # ---- Phases B + C + D ----
        with (
            tc.tile_pool(name=f"psS{rep}", bufs=1, space="PSUM") as psS,
            tc.tile_pool(name=f"psO{rep}", bufs=1, space="PSUM") as psO,
            tc.tile_pool(name=f"psP{rep}", bufs=2, space="PSUM") as psP,
            tc.tile_pool(name=f"csb{rep}", bufs=2) as csb,
        ):
            # -- projection fill units (psP double-buffered, 1 bank each) --
            def proj_v(kt):
                pv = psP.tile([P, NQ], F32, tag="pj")
                ksl = slice(kt * P, (kt + 1) * P)
                for c in range(DC):
                    nc.tensor.matmul(
                        pv[:, 0:INNER_L], xT[:, c, ksl], wv_sb[:, c, :],
                        start=(c == 0), stop=(c == DC - 1),
                    )
                nc.vector.tensor_copy(
                    V[:, kt, :, 0:DH],
                    pv[:, 0:INNER_L].rearrange("p (h d) -> p h d", h=HL),
                )

            def proj_q(hp, qc):
                pp = psP.tile([P, NQ], F32, tag="pj")
                qsl = slice(qc * NQ, (qc + 1) * NQ)
                for c in range(DC):
                    nc.tensor.matmul(
                        pp, wq_sb[:, c, hp * P:(hp + 1) * P], xT[:, c, qsl],
                        start=(c == 0), stop=(c == DC - 1),
                    )
                nc.vector.tensor_copy(QT[:, hp, qsl], pp)

            def proj_k(hp, qc):
                pp = psP.tile([P, NQ], F32, tag="pj")
                qsl = slice(qc * NQ, (qc + 1) * NQ)
                for c in range(DC):
                    nc.tensor.matmul(
                        pp, wk_sb[:, c, hp * P:(hp + 1) * P], xT[:, c, qsl],
                        start=(c == 0), stop=(c == DC - 1),
                    )
                kt0 = qc * 4
                nc.vector.tensor_copy(
                    KS[0:DH, hp, kt0:kt0 + 4, 0, :],
                    pp[0:DH, :].rearrange("p (t m) -> p t m", t=4),
                )
                nc.vector.tensor_copy(
                    KS[DH:P, hp, kt0:kt0 + 4, 1, :],
                    pp[DH:P, :].rearrange("p (t m) -> p t m", t=4),
                )

            def proj_out(kt):
                tsl = slice(kt * P, (kt + 1) * P)
                ost = csb.tile([P, D], F32, tag="ost")
                for nh in range(2):
                    po = psP.tile([P, NQ], F32, tag="pj")
                    for c in range(IC):
                        nc.tensor.matmul(
                            po[:, 0:INNER_L],
                            HT[:, c, tsl],
                            wo_sb[:, c, nh * INNER_L:(nh + 1) * INNER_L],
                            start=(c == 0), stop=(c == IC - 1),
                        )
                    nc.vector.tensor_copy(
                        ost[:, nh * INNER_L:(nh + 1) * INNER_L],
                        po[:, 0:INNER_L],
                    )
                nc.sync.dma_start(out_d[tsl, :], ost)

            # -- emit projections needed before attention --
            for hp_qc in range(QC):
                proj_q(0, hp_qc)
                proj_k(0, hp_qc)
            for kt in range(NT):
                proj_v(kt)

            # remaining fill units, popped between attention iterations.
            # K before Q: attention for hp needs ALL of KS[:, hp] but only
            # the current qc's QT slice.
            fills = [(proj_k, 1, qc) for qc in range(QC)]
            fills += [(proj_q, 1, qc) for qc in range(QC)]
            fills += [(proj_k, 2, qc) for qc in range(QC)]
            fills += [(proj_q, 2, qc) for qc in range(QC)]

            # -- attention: hp-major so hp+1 projections have time --
            for hp in range(IC):
                for qc in range(QC):
                    qsl = slice(qc * NQ, (qc + 1) * NQ)
                    o_e = psO.tile([DH + 1, NQ], F32, tag="oe")
                    o_o = psO.tile([DH + 1, NQ], F32, tag="oo")
                    o_ps = {0: o_e, 1: o_o}
                    p_prev = {0: None, 1: None}
                    for g in range(NG):
                        for eo in range(2):
                            s = psS.tile([P, G, NQ], F32,
                                         tag=("se", "so")[eo])
                            # interleave: scores for this group with AV of the
                            # previous group of the same head
                            pv = p_prev[eo]
                            for j in range(G):
                                kt = g * G + j
                                nc.tensor.matmul(
                                    s[:, j], KS[:, hp, kt, eo, :],
                                    QT[:, hp, qsl], start=True, stop=True,
                                )
                                if pv is not None:
                                    pkt = (g - 1) * G + j
                                    nc.tensor.matmul(
                                        o_ps[eo],
                                        V[:, pkt, 2 * hp + eo, :],
                                        pv[:, j],
                                        start=(pkt == 0), stop=False,
                                    )
                            p = csb.tile([P, G, NQ], BF,
                                         tag=("pe", "po")[eo], bufs=4)
                            nc.scalar.activation(
                                p, s, mybir.ActivationFunctionType.Exp,
                                scale=0.125,
                            )
                            p_prev[eo] = p
                    # drain final group's AV
                    for eo in range(2):
                        pv = p_prev[eo]
                        for j in range(G):
                            pkt = (NG - 1) * G + j
                            nc.tensor.matmul(
                                o_ps[eo], V[:, pkt, 2 * hp + eo, :], pv[:, j],
                                start=False, stop=(pkt == NT - 1),
                            )
                    # epilogue: normalize straight out of PSUM
                    for eo in range(2):
                        o = o_ps[eo]
                        rec = csb.tile([1, NQ], F32, tag="rec")
                        nc.vector.tensor_copy(rec, o[DH:DH + 1, :])
                        nc.vector.reciprocal(rec, rec)
                        rbc = csb.tile([DH, NQ], F32, tag=("rbce", "rbco")[eo])
                        nc.gpsimd.partition_broadcast(rbc, rec)
                        nc.vector.tensor_mul(
                            HT[eo * DH:(eo + 1) * DH, hp, qsl],
                            o[0:DH, :], rbc,
                        )
                    # pop projection fill units between iterations
                    for _ in range(2):
                        if fills:
                            fn, fhp, fqc = fills.pop(0)
                            fn(fhp, fqc)

            # ---- Phase D: output projection ----
            for kt in range(NT):
                proj_out(kt)


def _get_nc():
    global _CACHED_NC
    if _CACHED_NC is None:
        _CACHED_NC = build_program()
    return _CACHED_NC


def kernel(x, w_q, w_k, w_v, w_o, b_o):
    from concourse.bass_utils import run_bass_kernel_spmd

    x = np.asarray(x, dtype=np.float32)
    w_q = np.asarray(w_q, dtype=np.float32)
    w_k = np.asarray(w_k, dtype=np.float32)
    w_v = np.asarray(w_v, dtype=np.float32)
    w_o = np.asarray(w_o, dtype=np.float32)
    b_o = np.asarray(b_o, dtype=np.float32)

    nc = _get_nc()
    in_maps = []
    for c in range(8):
        b = c // 2
        s = slice((c % 2) * INNER_L, (c % 2) * INNER_L + INNER_L)
        in_maps.append({
            "x": np.ascontiguousarray(x[b]),
            "w_q": np.ascontiguousarray(w_q[:, s]),
            "w_k": np.ascontiguousarray(w_k[:, s]),
            "w_v": np.ascontiguousarray(w_v[:, s]),
            "w_o": np.ascontiguousarray(w_o[s, :]),
        })
    res = run_bass_kernel_spmd(nc, in_maps, list(range(8)))
    out = np.zeros((B, N, D), np.float32)
    for c in range(8):
        out[c // 2] += res.results[c]["out"]
    out += b_o
    return out


if __name__ == "__main__":
    rng = np.random.default_rng(0)
    ins = {
        "x": rng.standard_normal((B, N, D), dtype=np.float32),
        "w_q": (rng.standard_normal((D, D), dtype=np.float32) * 0.02),
        "w_k": (rng.standard_normal((D, D), dtype=np.float32) * 0.02),
        "w_v": (rng.standard_normal((D, D), dtype=np.float32) * 0.02),
        "w_o": (rng.standard_normal((D, D), dtype=np.float32) * 0.02),
        "b_o": np.zeros((D,), np.float32),
    }
    got = kernel(**ins)

    def ref(x, w_q, w_k, w_v, w_o, b_o):
        q = (x @ w_q).reshape(B, N, HEADS, DH).transpose(0, 2, 1, 3)
        k = (x @ w_k).reshape(B, N, HEADS, DH).transpose(0, 2, 1, 3)
        v = (x @ w_v).reshape(B, N, HEADS, DH).transpose(0, 2, 1, 3)
        s = np.einsum("bhnd,bhmd->bhnm", q, k) / 8.0
        s = s - s.max(axis=-1, keepdims=True)
        p = np.exp(s)
        p = p / p.sum(axis=-1, keepdims=True)
        h = np.einsum("bhnm,bhmd->bhnd", p, v)
        H = h.transpose(0, 2, 1, 3).reshape(B, N, HEADS * DH)
        return H @ w_o + b_o

    exp = ref(**ins)
    err = np.abs(got - exp)
    print(f"absmax err {err.max():.3e}  scale {np.abs(exp).max():.3e}  "
          f"rel {err.max() / np.abs(exp).max():.3e}")
